# revision 37
# baseline (speedup 1.0000x reference)
"""Trainium2 Bass kernel for nn_MultiHeadAttention (B=2, S=2048, D=1024, H=16).

Sharding: pure data-parallel over (batch, query-block). Core c handles batch
c//4 and query rows [512*(c%4), 512*(c%4+1)). Every core computes all 16 heads
for its 512 query rows, so there are no collectives; K/V projections are
duplicated across the 4 cores of a batch (PE has headroom under the HBM
roofline, which is dominated by the mandatory 512 MB attn output).

Math layout per core (all matmuls bf16, accumulation f32 in PSUM):
  - Host pre-transposes activations (x^T) so the contraction dim lands on
    SBUF partitions without any on-device transpose.
  - Scores are computed transposed: S^T[k, q] tiles, so exp can stream
    PSUM->SBUF on the scalar engine and the softmax denominator comes free
    from the attn@V matmul via a ones-column appended to V.
  - Masking is multiplicative after exp: exp(s)*keep, keep in {0,1}.
  - attn (normalized, f32) is written transposed [h, k, q]; the host
    transposes back while unsharding.
"""

import sys

sys.path.insert(0, "/opt/trn_rl_repo")

from contextlib import ExitStack

import ml_dtypes
import numpy as np

import concourse.bass as bass
import concourse.mybir as mybir
import concourse.tile as tile
from concourse import bacc

BF16 = mybir.dt.bfloat16
F32 = mybir.dt.float32
FP8 = mybir.dt.float8e4
MASK_C = -240.0

B, S, D = 2, 2048, 1024
H, DK = 16, 64
SQ = 512          # query rows per core
N_CORES = 8
KC = S // 128     # 16 k-chunks
IC = D // 128     # 8 contraction chunks
LN_EPS = 1e-5


def build():
    """Build the SPMD single-core program (identical on all 8 cores)."""
    nc = bacc.Bacc("TRN2", target_bir_lowering=False, debug=False,
                   num_devices=N_CORES)

    xqT = nc.dram_tensor("xqT", [D, SQ], BF16, kind="ExternalInput").ap()
    xkT = nc.dram_tensor("xkT", [D, S], BF16, kind="ExternalInput").ap()
    xvT = nc.dram_tensor("xvT", [D, S], BF16, kind="ExternalInput").ap()
    xq_res = nc.dram_tensor("xq_res", [SQ, D], F32, kind="ExternalInput").ap()
    maskT = nc.dram_tensor("maskT", [S, SQ], FP8, kind="ExternalInput").ap()
    negid = nc.dram_tensor("negid", [128, 128], FP8, kind="ExternalInput").ap()
    w_q = nc.dram_tensor("w_q", [D, D], BF16, kind="ExternalInput").ap()
    w_k = nc.dram_tensor("w_k", [D, D], BF16, kind="ExternalInput").ap()
    w_v = nc.dram_tensor("w_v", [D, D], BF16, kind="ExternalInput").ap()
    w_o = nc.dram_tensor("w_o", [D, D], BF16, kind="ExternalInput").ap()
    attn_t = nc.dram_tensor("attn_t", [H, S, SQ], F32, kind="ExternalOutput").ap()
    y_out = nc.dram_tensor("y_out", [SQ, D], F32, kind="ExternalOutput").ap()

    with tile.TileContext(nc) as tc, ExitStack() as ctx:
        res = ctx.enter_context(tc.tile_pool(name="res", bufs=1))
        psum = ctx.enter_context(tc.tile_pool(name="psum", bufs=1, space="PSUM"))

        # ---- persistent SBUF tensors (~105 KB/partition) ----
        qt_sb = res.tile([128, IC, SQ], BF16, tag="qt")     # Q^T [o, q]
        kt_sb = res.tile([128, IC, S], BF16, tag="kt")      # K^T [o, k]
        v_sb = res.tile([128, KC, H * 65], BF16, tag="v")   # V+ones [k, h*65+d]
        ctxn_sb = res.tile([128, IC, SQ], BF16, tag="ctxn")  # ctx^T [c, q]
        negid_sb = res.tile([128, 128], FP8, tag="negid")   # MASK_C * I
        wo_sb = res.tile([128, IC, D], BF16, tag="wo")
        xr_sb = res.tile([128, SQ // 128, D], F32, tag="xr")
        eps_sb = res.tile([128, 1], F32, tag="eps")

        # ---- phase 1: projections ----
        with tc.tile_pool(name="p1", bufs=1) as p1:
            xq_sb = p1.tile([128, IC, SQ], BF16, tag="xq")
            xk_sb = p1.tile([128, IC, S], BF16, tag="xk")
            wk_sb = p1.tile([128, IC, D], BF16, tag="wk")
            wv_sb = p1.tile([128, IC, D], BF16, tag="wv")
            # DMA order: the Q/K inputs the first matmuls need come first.
            nc.sync.dma_start(xq_sb[:], xqT.rearrange("(ic p) q -> p ic q", p=128))
            for nt in range(4):
                nc.sync.dma_start(
                    xk_sb[:, :, nt * 512:(nt + 1) * 512],
                    xkT[:, nt * 512:(nt + 1) * 512]
                    .rearrange("(ic p) k -> p ic k", p=128))
            for oc in range(4):
                nc.sync.dma_start(
                    wk_sb[:, :, oc * 256:(oc + 1) * 256],
                    w_k[:, oc * 256:(oc + 1) * 256]
                    .rearrange("(ic p) o -> p ic o", p=128))
            # ones for the V ones-column (V copies overwrite the rest)
            nc.vector.memset(v_sb[:], 1.0)
            nc.vector.memset(eps_sb[:], LN_EPS)

            # Q^T and K^T per head-pair (pair 0 finishes first so attention
            # can start while later pairs still project).
            for hpp in range(IC // 2):
                wq_t = p1.tile([128, IC, 256], BF16, tag="wq", bufs=2)
                nc.sync.dma_start(
                    wq_t[:],
                    w_q[:, hpp * 256:(hpp + 1) * 256]
                    .rearrange("(ic p) o -> p ic o", p=128))
                ps_q = psum.tile([128, 1024], F32, tag="mm2", bufs=3)
                for j2 in range(2):
                    for ic in range(IC):
                        nc.tensor.matmul(ps_q[:, j2 * 512:(j2 + 1) * 512],
                                         lhsT=wq_t[:, ic, j2 * 128:(j2 + 1) * 128],
                                         rhs=xq_sb[:, ic, :],
                                         start=(ic == 0), stop=(ic == IC - 1))
                nc.scalar.copy(qt_sb[:, 2 * hpp:2 * hpp + 2, :],
                               ps_q[:].rearrange("p (x q) -> p x q", x=2))
                for nt in range(S // 512):
                    ps_k = psum.tile([128, 1024], F32, tag="mm2", bufs=3)
                    for j2 in range(2):
                        hp = 2 * hpp + j2
                        for ic in range(IC):
                            nc.tensor.matmul(
                                ps_k[:, j2 * 512:(j2 + 1) * 512],
                                lhsT=wk_sb[:, ic, hp * 128:(hp + 1) * 128],
                                rhs=xk_sb[:, ic, nt * 512:(nt + 1) * 512],
                                start=(ic == 0), stop=(ic == IC - 1))
                    nc.scalar.copy(
                        kt_sb[:, 2 * hpp:2 * hpp + 2, nt * 512:(nt + 1) * 512],
                        ps_k[:].rearrange("p (x q) -> p x q", x=2))

            nc.sync.dma_start(negid_sb[:], negid)
            nc.sync.dma_start(wv_sb[:], w_v.rearrange("(ic p) o -> p ic o", p=128))
            nc.sync.dma_start(wo_sb[:], w_o.rearrange("(ic p) o -> p ic o", p=128))
            nc.sync.dma_start(xr_sb[:], xq_res.rearrange("(qc p) o -> p qc o", p=128))

            # V[k, o]: lhsT = x_v^T k-block (streamed), rhs = W_V. Strided copy
            # into v_sb leaves the per-head ones-columns (h*65+64) intact.
            for kc in range(KC):
                xv_t = p1.tile([128, IC, 128], BF16, tag="xv", bufs=2)
                nc.sync.dma_start(
                    xv_t[:],
                    xvT[:, kc * 128:(kc + 1) * 128]
                    .rearrange("(ic p) k -> p ic k", p=128))
                ps_v = psum.tile([128, 1024], F32, tag="mm2", bufs=3)
                for half in range(2):
                    for ic in range(IC):
                        nc.tensor.matmul(ps_v[:, half * 512:(half + 1) * 512],
                                         lhsT=xv_t[:, ic, :],
                                         rhs=wv_sb[:, ic, half * 512:(half + 1) * 512],
                                         start=(ic == 0), stop=(ic == IC - 1))
                dst = v_sb[:, kc, :].rearrange("p (h x) -> p h x", x=65)
                nc.scalar.copy(dst[:, :, 0:64],
                               ps_v[:].rearrange("p (h d) -> p h d", d=64))

        # ---- phase 2: attention, head pairs (row-packed K=64 matmuls) ----
        # Scores go into 2-bank PSUM tiles; the mask is applied by a second
        # accumulating matmul with lhsT = -1e9*I and rhs = maskT, so exp can
        # read fully-masked scores and no elementwise mask op is needed.
        with tc.tile_pool(name="p2", bufs=1) as p2:
            mask_sb = p2.tile([128, KC, SQ], FP8, tag="mask")  # 1.0 = blocked
            nc.sync.dma_start(mask_sb[:],
                              maskT.rearrange("(kc p) q -> p kc q", p=128))
            for hp in range(IC):
                heads = (2 * hp, 2 * hp + 1)
                masked = [p2.tile([128, KC, SQ], BF16, tag="masked", bufs=4,
                                  name=f"masked_{h}") for h in heads]
                for kcg in range(KC // 2):
                    ps_s = [psum.tile([128, 1024], F32, tag="mm2", bufs=3,
                                      name=f"ps_s_{j}") for j in range(2)]
                    # the two K=64 QK matmuls sit in different PE row groups
                    # and run concurrently when issued back-to-back
                    for t in range(2):
                        kc = 2 * kcg + t
                        for j in range(2):
                            p0 = j * 64
                            nc.tensor.matmul(
                                ps_s[j][:, t * 512:(t + 1) * 512],
                                lhsT=kt_sb[p0:p0 + 64, hp,
                                           kc * 128:(kc + 1) * 128],
                                rhs=qt_sb[p0:p0 + 64, hp, :],
                                start=True, stop=False)
                    for t in range(2):
                        kc = 2 * kcg + t
                        for j in range(2):
                            nc.tensor.matmul(
                                ps_s[j][:, t * 512:(t + 1) * 512],
                                lhsT=negid_sb[:],
                                rhs=mask_sb[:, kc, :],
                                start=False, stop=True)
                    for j in range(2):
                        nc.scalar.activation(
                            masked[j][:, 2 * kcg:2 * kcg + 2, :], ps_s[j][:],
                            mybir.ActivationFunctionType.Exp, scale=0.125)
                # attn @ [V | 1]: ctx^T rows 0..63, denominators in row 64.
                # Both heads share one 2-bank psum tile so recip + broadcast
                # run once per pair.
                pc = psum.tile([65, 1024], F32, tag="ctx", bufs=1)
                for j, h in enumerate(heads):
                    for kc in range(KC):
                        nc.tensor.matmul(pc[:, j * 512:(j + 1) * 512],
                                         lhsT=v_sb[:, kc, h * 65:(h + 1) * 65],
                                         rhs=masked[j][:, kc, :],
                                         start=(kc == 0), stop=(kc == KC - 1))
                recip = p2.tile([1, 1024], BF16, tag="recip", bufs=1)
                with nc.allow_low_precision(reason="softmax denom recip to bf16"):
                    nc.vector.reciprocal(recip[:], pc[64:65, :])
                bcast = p2.tile([128, 1024], BF16, tag="bcast", bufs=2)
                nc.gpsimd.partition_broadcast(bcast[:], recip[:])
                for j, h in enumerate(heads):
                    p0 = j * 64
                    sl = slice(j * 512, (j + 1) * 512)
                    nc.vector.tensor_tensor(ctxn_sb[p0:p0 + 64, hp, :],
                                            pc[0:64, sl], bcast[0:64, sl],
                                            op=mybir.AluOpType.mult)
                    bc4 = bcast[:, sl].rearrange("p (x q) -> p x q", x=1) \
                        .to_broadcast((128, 4, 512))
                    for g in range(4):
                        at = p2.tile([128, 4, SQ], F32, tag="attn_out", bufs=2)
                        nc.vector.tensor_tensor(
                            at[:], masked[j][:, g * 4:(g + 1) * 4, :], bc4,
                            op=mybir.AluOpType.mult)
                        nc.sync.dma_start(
                            attn_t[h, g * 512:(g + 1) * 512, :]
                            .rearrange("(x p) q -> p x q", p=128),
                            at[:])

        # ---- phase 3: output projection + residual + layernorm ----
        with tc.tile_pool(name="p3", bufs=1) as p3:
            for qc in range(SQ // 128):
                ps_y = psum.tile([128, 1024], F32, tag="mm2", bufs=3)
                for half in range(2):
                    for cc in range(IC):
                        nc.tensor.matmul(
                            ps_y[:, half * 512:(half + 1) * 512],
                            lhsT=ctxn_sb[:, cc, qc * 128:(qc + 1) * 128],
                            rhs=wo_sb[:, cc, half * 512:(half + 1) * 512],
                            start=(cc == 0), stop=(cc == IC - 1))
                y_sb = p3.tile([128, D], F32, tag="y", bufs=2)
                nc.vector.tensor_add(y_sb[:], ps_y[:], xr_sb[:, qc, :])
                nmean = p3.tile([128, 1], F32, tag="nmean", bufs=2)
                nc.vector.reduce_sum(nmean[:], y_sb[:], axis=mybir.AxisListType.X)
                nc.vector.tensor_scalar_mul(nmean[:], nmean[:], -1.0 / D)
                nc.vector.tensor_scalar_add(y_sb[:], y_sb[:], nmean[:])
                sq_ps = psum.tile([128, 1024], F32, tag="mm2", bufs=3)
                var = p3.tile([128, 1], F32, tag="var", bufs=2)
                nc.scalar.activation(sq_ps[:], y_sb[:],
                                     mybir.ActivationFunctionType.Square,
                                     accum_out=var[:])
                std = p3.tile([128, 1], F32, tag="std", bufs=2)
                nc.scalar.activation(std[:], var[:],
                                     mybir.ActivationFunctionType.Sqrt,
                                     bias=eps_sb[:], scale=1.0 / D)
                rstd = p3.tile([128, 1], F32, tag="rstd", bufs=2)
                nc.vector.reciprocal(rstd[:], std[:])
                o_sb = p3.tile([128, D], F32, tag="o", bufs=2)
                nc.vector.tensor_scalar_mul(o_sb[:], y_sb[:], rstd[:])
                nc.sync.dma_start(y_out[qc * 128:(qc + 1) * 128, :], o_sb[:])

    nc.compile()
    return nc


_NC_CACHE = None


def get_nc():
    global _NC_CACHE
    if _NC_CACHE is None:
        _NC_CACHE = build()
    return _NC_CACHE


def make_in_maps(input_Q, input_K, input_V, attn_mask, W_Q, W_K, W_V, W_O):
    bf = ml_dtypes.bfloat16
    input_Q = np.asarray(input_Q, np.float32)
    input_K = np.asarray(input_K, np.float32)
    input_V = np.asarray(input_V, np.float32)
    attn_mask = np.asarray(attn_mask, bool)
    shared = {
        "w_q": np.asarray(W_Q, np.float32).astype(bf),
        "w_k": np.asarray(W_K, np.float32).astype(bf),
        "w_v": np.asarray(W_V, np.float32).astype(bf),
        "w_o": np.asarray(W_O, np.float32).astype(bf),
        "negid": (np.eye(128, dtype=np.float32) * MASK_C).astype(
            ml_dtypes.float8_e4m3fn),
    }
    per_batch = {}
    for b in range(B):
        per_batch[b] = {
            "xkT": np.ascontiguousarray(input_K[b].T).astype(bf),
            "xvT": np.ascontiguousarray(input_V[b].T).astype(bf),
        }
    in_maps = []
    for c in range(N_CORES):
        b, qi = divmod(c, 4)
        q0 = qi * SQ
        xq = input_Q[b, q0:q0 + SQ, :]
        in_maps.append({
            **shared,
            **per_batch[b],
            "xqT": np.ascontiguousarray(xq.T).astype(bf),
            "xq_res": np.ascontiguousarray(xq),
            "maskT": np.ascontiguousarray(
                attn_mask[b, q0:q0 + SQ, :].T.astype(np.float32)).astype(
                    ml_dtypes.float8_e4m3fn),
        })
    return in_maps


def assemble(results):
    out = np.empty((B, S, D), np.float32)
    attn = np.empty((B, H, S, S), np.float32)
    for c in range(N_CORES):
        b, qi = divmod(c, 4)
        q0 = qi * SQ
        out[b, q0:q0 + SQ] = results[c]["y_out"]
        attn[b, :, q0:q0 + SQ, :] = results[c]["attn_t"].transpose(0, 2, 1)
    return out, attn


def run(inputs, trace=False, trace_kwargs=None):
    from concourse.bass_utils import run_bass_kernel_spmd
    nc = get_nc()
    in_maps = make_in_maps(**inputs)
    res = run_bass_kernel_spmd(nc, in_maps, core_ids=list(range(N_CORES)),
                               trace=trace, **(trace_kwargs or {}))
    return res


def kernel(**inputs):
    res = run(inputs, trace=False)
    return assemble(res.results)


# revision 38
# speedup vs baseline: 1.0794x; 1.0794x over previous
"""Trainium2 Bass kernel for nn_MultiHeadAttention (B=2, S=2048, D=1024, H=16).

Sharding: pure data-parallel over (batch, query-block). Core c handles batch
c//4 and query rows [512*(c%4), 512*(c%4+1)). Every core computes all 16 heads
for its 512 query rows, so there are no collectives; K/V projections are
duplicated across the 4 cores of a batch (PE has headroom under the HBM
roofline, which is dominated by the mandatory 512 MB attn output).

Math layout per core (all matmuls bf16, accumulation f32 in PSUM):
  - Host pre-transposes activations (x^T) so the contraction dim lands on
    SBUF partitions without any on-device transpose.
  - Scores are computed transposed: S^T[k, q] tiles, so exp can stream
    PSUM->SBUF on the scalar engine and the softmax denominator comes free
    from the attn@V matmul via a ones-column appended to V.
  - Masking is multiplicative after exp: exp(s)*keep, keep in {0,1}.
  - attn (normalized, f32) is written transposed [h, k, q]; the host
    transposes back while unsharding.
"""

import sys

sys.path.insert(0, "/opt/trn_rl_repo")

from contextlib import ExitStack

import ml_dtypes
import numpy as np

import concourse.bass as bass
import concourse.mybir as mybir
import concourse.tile as tile
from concourse import bacc

BF16 = mybir.dt.bfloat16
F32 = mybir.dt.float32
FP8 = mybir.dt.float8e4
MASK_C = -240.0

B, S, D = 2, 2048, 1024
H, DK = 16, 64
SQ = 512          # query rows per core
N_CORES = 8
KC = S // 128     # 16 k-chunks
IC = D // 128     # 8 contraction chunks
LN_EPS = 1e-5


def build():
    """Build the SPMD single-core program (identical on all 8 cores)."""
    nc = bacc.Bacc("TRN2", target_bir_lowering=False, debug=False,
                   num_devices=N_CORES)

    xqT = nc.dram_tensor("xqT", [D, SQ], BF16, kind="ExternalInput").ap()
    xkT = nc.dram_tensor("xkT", [D, S], BF16, kind="ExternalInput").ap()
    xvT = nc.dram_tensor("xvT", [D, S], BF16, kind="ExternalInput").ap()
    xq_res = nc.dram_tensor("xq_res", [SQ, D], F32, kind="ExternalInput").ap()
    maskT = nc.dram_tensor("maskT", [S, SQ], FP8, kind="ExternalInput").ap()
    negid = nc.dram_tensor("negid", [128, 128], FP8, kind="ExternalInput").ap()
    w_q = nc.dram_tensor("w_q", [D, D], BF16, kind="ExternalInput").ap()
    w_k = nc.dram_tensor("w_k", [D, D], BF16, kind="ExternalInput").ap()
    w_v = nc.dram_tensor("w_v", [D, D], BF16, kind="ExternalInput").ap()
    w_o = nc.dram_tensor("w_o", [D, D], BF16, kind="ExternalInput").ap()
    attn_t = nc.dram_tensor("attn_t", [H, S, SQ], F32, kind="ExternalOutput").ap()
    y_out = nc.dram_tensor("y_out", [SQ, D], F32, kind="ExternalOutput").ap()

    with tile.TileContext(nc) as tc, ExitStack() as ctx:
        res = ctx.enter_context(tc.tile_pool(name="res", bufs=1))
        psum = ctx.enter_context(tc.tile_pool(name="psum", bufs=1, space="PSUM"))

        # ---- persistent SBUF tensors (~105 KB/partition) ----
        qt_sb = res.tile([128, IC, SQ], BF16, tag="qt")     # Q^T [o, q]
        kt_sb = res.tile([128, IC, S], BF16, tag="kt")      # K^T [o, k]
        v_sb = res.tile([128, KC, H * 65], BF16, tag="v")   # V+ones [k, h*65+d]
        ctxn_sb = res.tile([128, IC, SQ], BF16, tag="ctxn")  # ctx^T [c, q]
        negid_sb = res.tile([128, 128], FP8, tag="negid")   # MASK_C * I
        wo_sb = res.tile([128, IC, D], BF16, tag="wo")
        xr_sb = res.tile([128, SQ // 128, D], F32, tag="xr")
        eps_sb = res.tile([128, 1], F32, tag="eps")

        # ---- phase 1: projections ----
        with tc.tile_pool(name="p1", bufs=1) as p1:
            xq_sb = p1.tile([128, IC, SQ], BF16, tag="xq")
            xk_sb = p1.tile([128, IC, S], BF16, tag="xk")
            wk_sb = p1.tile([128, IC, D], BF16, tag="wk")
            wv_sb = p1.tile([128, IC, D], BF16, tag="wv")
            # DMA order: the Q/K inputs the first matmuls need come first.
            nc.sync.dma_start(xq_sb[:], xqT.rearrange("(ic p) q -> p ic q", p=128))
            for nt in range(4):
                nc.sync.dma_start(
                    xk_sb[:, :, nt * 512:(nt + 1) * 512],
                    xkT[:, nt * 512:(nt + 1) * 512]
                    .rearrange("(ic p) k -> p ic k", p=128))
            for oc in range(4):
                nc.sync.dma_start(
                    wk_sb[:, :, oc * 256:(oc + 1) * 256],
                    w_k[:, oc * 256:(oc + 1) * 256]
                    .rearrange("(ic p) o -> p ic o", p=128))
            # ones for the V ones-column (V copies overwrite the rest)
            nc.vector.memset(v_sb[:], 1.0)
            nc.vector.memset(eps_sb[:], LN_EPS)

            # Q^T and K^T per head-pair (pair 0 finishes first so attention
            # can start while later pairs still project).
            for hpp in range(IC // 2):
                wq_t = p1.tile([128, IC, 256], BF16, tag="wq", bufs=2)
                nc.sync.dma_start(
                    wq_t[:],
                    w_q[:, hpp * 256:(hpp + 1) * 256]
                    .rearrange("(ic p) o -> p ic o", p=128))
                ps_q = psum.tile([128, 1024], F32, tag="mm2", bufs=3)
                for j2 in range(2):
                    for ic in range(IC):
                        nc.tensor.matmul(ps_q[:, j2 * 512:(j2 + 1) * 512],
                                         lhsT=wq_t[:, ic, j2 * 128:(j2 + 1) * 128],
                                         rhs=xq_sb[:, ic, :],
                                         start=(ic == 0), stop=(ic == IC - 1))
                nc.scalar.copy(qt_sb[:, 2 * hpp:2 * hpp + 2, :],
                               ps_q[:].rearrange("p (x q) -> p x q", x=2))
                for nt in range(S // 512):
                    ps_k = psum.tile([128, 1024], F32, tag="mm2", bufs=3)
                    for j2 in range(2):
                        hp = 2 * hpp + j2
                        for ic in range(IC):
                            nc.tensor.matmul(
                                ps_k[:, j2 * 512:(j2 + 1) * 512],
                                lhsT=wk_sb[:, ic, hp * 128:(hp + 1) * 128],
                                rhs=xk_sb[:, ic, nt * 512:(nt + 1) * 512],
                                start=(ic == 0), stop=(ic == IC - 1))
                    nc.scalar.copy(
                        kt_sb[:, 2 * hpp:2 * hpp + 2, nt * 512:(nt + 1) * 512],
                        ps_k[:].rearrange("p (x q) -> p x q", x=2))

            nc.sync.dma_start(negid_sb[:], negid)
            nc.sync.dma_start(wv_sb[:], w_v.rearrange("(ic p) o -> p ic o", p=128))
            nc.sync.dma_start(wo_sb[:], w_o.rearrange("(ic p) o -> p ic o", p=128))
            nc.sync.dma_start(xr_sb[:], xq_res.rearrange("(qc p) o -> p qc o", p=128))

            # V[k, o]: lhsT = x_v^T k-block (streamed), rhs = W_V. Strided copy
            # into v_sb leaves the per-head ones-columns (h*65+64) intact.
            for kc in range(KC):
                xv_t = p1.tile([128, IC, 128], BF16, tag="xv", bufs=2)
                nc.sync.dma_start(
                    xv_t[:],
                    xvT[:, kc * 128:(kc + 1) * 128]
                    .rearrange("(ic p) k -> p ic k", p=128))
                ps_v = psum.tile([128, 1024], F32, tag="mm2", bufs=3)
                for half in range(2):
                    for ic in range(IC):
                        nc.tensor.matmul(ps_v[:, half * 512:(half + 1) * 512],
                                         lhsT=xv_t[:, ic, :],
                                         rhs=wv_sb[:, ic, half * 512:(half + 1) * 512],
                                         start=(ic == 0), stop=(ic == IC - 1))
                dst = v_sb[:, kc, :].rearrange("p (h x) -> p h x", x=65)
                nc.scalar.copy(dst[:, :, 0:64],
                               ps_v[:].rearrange("p (h d) -> p h d", d=64))

        # ---- phase 2: attention, head pairs (row-packed K=64 matmuls) ----
        # Scores go into 2-bank PSUM tiles; the mask is applied by a second
        # accumulating matmul with lhsT = -1e9*I and rhs = maskT, so exp can
        # read fully-masked scores and no elementwise mask op is needed.
        with tc.tile_pool(name="p2", bufs=1) as p2:
            mask_sb = p2.tile([128, KC, SQ], FP8, tag="mask")  # 1.0 = blocked
            nc.sync.dma_start(mask_sb[:],
                              maskT.rearrange("(kc p) q -> p kc q", p=128))
            for hp in range(IC):
                heads = (2 * hp, 2 * hp + 1)
                masked = [p2.tile([128, KC, SQ], BF16, tag="masked", bufs=3,
                                  name=f"masked_{h}") for h in heads]
                for kcg in range(KC // 2):
                    ps_s = [psum.tile([128, 1024], F32, tag="mm2", bufs=3,
                                      name=f"ps_s_{j}") for j in range(2)]
                    # the two K=64 QK matmuls sit in different PE row groups
                    # and run concurrently when issued back-to-back
                    for t in range(2):
                        kc = 2 * kcg + t
                        for j in range(2):
                            p0 = j * 64
                            nc.tensor.matmul(
                                ps_s[j][:, t * 512:(t + 1) * 512],
                                lhsT=kt_sb[p0:p0 + 64, hp,
                                           kc * 128:(kc + 1) * 128],
                                rhs=qt_sb[p0:p0 + 64, hp, :],
                                start=True, stop=False)
                    for t in range(2):
                        kc = 2 * kcg + t
                        for j in range(2):
                            nc.tensor.matmul(
                                ps_s[j][:, t * 512:(t + 1) * 512],
                                lhsT=negid_sb[:],
                                rhs=mask_sb[:, kc, :],
                                start=False, stop=True)
                    for j in range(2):
                        nc.scalar.activation(
                            masked[j][:, 2 * kcg:2 * kcg + 2, :], ps_s[j][:],
                            mybir.ActivationFunctionType.Exp, scale=0.125)
                # attn @ [V | 1]: ctx^T rows 0..63, denominators in row 64.
                # Both heads share one 2-bank psum tile so recip + broadcast
                # run once per pair.
                pc = psum.tile([65, 1024], F32, tag="ctx", bufs=1)
                for j, h in enumerate(heads):
                    for kc in range(KC):
                        nc.tensor.matmul(pc[:, j * 512:(j + 1) * 512],
                                         lhsT=v_sb[:, kc, h * 65:(h + 1) * 65],
                                         rhs=masked[j][:, kc, :],
                                         start=(kc == 0), stop=(kc == KC - 1))
                recip = p2.tile([1, 1024], BF16, tag="recip", bufs=2)
                with nc.allow_low_precision(reason="softmax denom recip to bf16"):
                    nc.vector.reciprocal(recip[:], pc[64:65, :])
                bcast = p2.tile([128, 1024], BF16, tag="bcast", bufs=2)
                nc.gpsimd.partition_broadcast(bcast[:], recip[:])
                for j, h in enumerate(heads):
                    p0 = j * 64
                    sl = slice(j * 512, (j + 1) * 512)
                    nc.vector.tensor_tensor(ctxn_sb[p0:p0 + 64, hp, :],
                                            pc[0:64, sl], bcast[0:64, sl],
                                            op=mybir.AluOpType.mult)
                    bc4 = bcast[:, sl].rearrange("p (x q) -> p x q", x=1) \
                        .to_broadcast((128, 4, 512))
                    for g in range(4):
                        at = p2.tile([128, 4, SQ], F32, tag="attn_out", bufs=2)
                        nc.vector.tensor_tensor(
                            at[:], masked[j][:, g * 4:(g + 1) * 4, :], bc4,
                            op=mybir.AluOpType.mult)
                        nc.sync.dma_start(
                            attn_t[h, g * 512:(g + 1) * 512, :]
                            .rearrange("(x p) q -> p x q", p=128),
                            at[:])

        # ---- phase 3: output projection + residual + layernorm ----
        with tc.tile_pool(name="p3", bufs=1) as p3:
            for qc in range(SQ // 128):
                ps_y = psum.tile([128, 1024], F32, tag="mm2", bufs=3)
                for half in range(2):
                    for cc in range(IC):
                        nc.tensor.matmul(
                            ps_y[:, half * 512:(half + 1) * 512],
                            lhsT=ctxn_sb[:, cc, qc * 128:(qc + 1) * 128],
                            rhs=wo_sb[:, cc, half * 512:(half + 1) * 512],
                            start=(cc == 0), stop=(cc == IC - 1))
                y_sb = p3.tile([128, D], F32, tag="y", bufs=2)
                nc.vector.tensor_add(y_sb[:], ps_y[:], xr_sb[:, qc, :])
                nmean = p3.tile([128, 1], F32, tag="nmean", bufs=2)
                nc.vector.reduce_sum(nmean[:], y_sb[:], axis=mybir.AxisListType.X)
                nc.vector.tensor_scalar_mul(nmean[:], nmean[:], -1.0 / D)
                nc.vector.tensor_scalar_add(y_sb[:], y_sb[:], nmean[:])
                sq_ps = psum.tile([128, 1024], F32, tag="mm2", bufs=3)
                var = p3.tile([128, 1], F32, tag="var", bufs=2)
                nc.scalar.activation(sq_ps[:], y_sb[:],
                                     mybir.ActivationFunctionType.Square,
                                     accum_out=var[:])
                std = p3.tile([128, 1], F32, tag="std", bufs=2)
                nc.scalar.activation(std[:], var[:],
                                     mybir.ActivationFunctionType.Sqrt,
                                     bias=eps_sb[:], scale=1.0 / D)
                rstd = p3.tile([128, 1], F32, tag="rstd", bufs=2)
                nc.vector.reciprocal(rstd[:], std[:])
                o_sb = p3.tile([128, D], F32, tag="o", bufs=2)
                nc.vector.tensor_scalar_mul(o_sb[:], y_sb[:], rstd[:])
                nc.sync.dma_start(y_out[qc * 128:(qc + 1) * 128, :], o_sb[:])

    nc.compile()
    return nc


_NC_CACHE = None


def get_nc():
    global _NC_CACHE
    if _NC_CACHE is None:
        _NC_CACHE = build()
    return _NC_CACHE


def make_in_maps(input_Q, input_K, input_V, attn_mask, W_Q, W_K, W_V, W_O):
    bf = ml_dtypes.bfloat16
    input_Q = np.asarray(input_Q, np.float32)
    input_K = np.asarray(input_K, np.float32)
    input_V = np.asarray(input_V, np.float32)
    attn_mask = np.asarray(attn_mask, bool)
    shared = {
        "w_q": np.asarray(W_Q, np.float32).astype(bf),
        "w_k": np.asarray(W_K, np.float32).astype(bf),
        "w_v": np.asarray(W_V, np.float32).astype(bf),
        "w_o": np.asarray(W_O, np.float32).astype(bf),
        "negid": (np.eye(128, dtype=np.float32) * MASK_C).astype(
            ml_dtypes.float8_e4m3fn),
    }
    per_batch = {}
    for b in range(B):
        per_batch[b] = {
            "xkT": np.ascontiguousarray(input_K[b].T).astype(bf),
            "xvT": np.ascontiguousarray(input_V[b].T).astype(bf),
        }
    in_maps = []
    for c in range(N_CORES):
        b, qi = divmod(c, 4)
        q0 = qi * SQ
        xq = input_Q[b, q0:q0 + SQ, :]
        in_maps.append({
            **shared,
            **per_batch[b],
            "xqT": np.ascontiguousarray(xq.T).astype(bf),
            "xq_res": np.ascontiguousarray(xq),
            "maskT": np.ascontiguousarray(
                attn_mask[b, q0:q0 + SQ, :].T.astype(np.float32)).astype(
                    ml_dtypes.float8_e4m3fn),
        })
    return in_maps


def assemble(results):
    out = np.empty((B, S, D), np.float32)
    attn = np.empty((B, H, S, S), np.float32)
    for c in range(N_CORES):
        b, qi = divmod(c, 4)
        q0 = qi * SQ
        out[b, q0:q0 + SQ] = results[c]["y_out"]
        attn[b, :, q0:q0 + SQ, :] = results[c]["attn_t"].transpose(0, 2, 1)
    return out, attn


def run(inputs, trace=False, trace_kwargs=None):
    from concourse.bass_utils import run_bass_kernel_spmd
    nc = get_nc()
    in_maps = make_in_maps(**inputs)
    res = run_bass_kernel_spmd(nc, in_maps, core_ids=list(range(N_CORES)),
                               trace=trace, **(trace_kwargs or {}))
    return res


def kernel(**inputs):
    res = run(inputs, trace=False)
    return assemble(res.results)


# revision 39
# speedup vs baseline: 1.1061x; 1.0247x over previous
"""Trainium2 Bass kernel for nn_MultiHeadAttention (B=2, S=2048, D=1024, H=16).

Sharding: pure data-parallel over (batch, query-block). Core c handles batch
c//4 and query rows [512*(c%4), 512*(c%4+1)). Every core computes all 16 heads
for its 512 query rows, so there are no collectives; K/V projections are
duplicated across the 4 cores of a batch (PE has headroom under the HBM
roofline, which is dominated by the mandatory 512 MB attn output).

Math layout per core (all matmuls bf16, accumulation f32 in PSUM):
  - Host pre-transposes activations (x^T) so the contraction dim lands on
    SBUF partitions without any on-device transpose.
  - Scores are computed transposed: S^T[k, q] tiles, so exp can stream
    PSUM->SBUF on the scalar engine and the softmax denominator comes free
    from the attn@V matmul via a ones-column appended to V.
  - Masking is multiplicative after exp: exp(s)*keep, keep in {0,1}.
  - attn (normalized, f32) is written transposed [h, k, q]; the host
    transposes back while unsharding.
"""

import sys

sys.path.insert(0, "/opt/trn_rl_repo")

from contextlib import ExitStack

import ml_dtypes
import numpy as np

import concourse.bass as bass
import concourse.mybir as mybir
import concourse.tile as tile
from concourse import bacc

BF16 = mybir.dt.bfloat16
F32 = mybir.dt.float32
FP8 = mybir.dt.float8e4
MASK_C = -240.0

B, S, D = 2, 2048, 1024
H, DK = 16, 64
SQ = 512          # query rows per core
N_CORES = 8
KC = S // 128     # 16 k-chunks
IC = D // 128     # 8 contraction chunks
LN_EPS = 1e-5


def build():
    """Build the SPMD single-core program (identical on all 8 cores)."""
    nc = bacc.Bacc("TRN2", target_bir_lowering=False, debug=False,
                   num_devices=N_CORES)

    xqT = nc.dram_tensor("xqT", [D, SQ], BF16, kind="ExternalInput").ap()
    xkT = nc.dram_tensor("xkT", [D, S], BF16, kind="ExternalInput").ap()
    xvT = nc.dram_tensor("xvT", [D, S], BF16, kind="ExternalInput").ap()
    xq_res = nc.dram_tensor("xq_res", [SQ, D], F32, kind="ExternalInput").ap()
    maskT = nc.dram_tensor("maskT", [S, SQ], FP8, kind="ExternalInput").ap()
    negid = nc.dram_tensor("negid", [128, 128], FP8, kind="ExternalInput").ap()
    w_q = nc.dram_tensor("w_q", [D, D], BF16, kind="ExternalInput").ap()
    w_k = nc.dram_tensor("w_k", [D, D], BF16, kind="ExternalInput").ap()
    w_v = nc.dram_tensor("w_v", [D, D], BF16, kind="ExternalInput").ap()
    w_o = nc.dram_tensor("w_o", [D, D], BF16, kind="ExternalInput").ap()
    attn_t = nc.dram_tensor("attn_t", [H, S, SQ], F32, kind="ExternalOutput").ap()
    y_out = nc.dram_tensor("y_out", [SQ, D], F32, kind="ExternalOutput").ap()

    with tile.TileContext(nc) as tc, ExitStack() as ctx:
        res = ctx.enter_context(tc.tile_pool(name="res", bufs=1))
        psum = ctx.enter_context(tc.tile_pool(name="psum", bufs=1, space="PSUM"))

        # ---- persistent SBUF tensors (~105 KB/partition) ----
        qt_sb = res.tile([128, IC, SQ], BF16, tag="qt")     # Q^T [o, q]
        kt_sb = res.tile([128, IC, S], BF16, tag="kt")      # K^T [o, k]
        v_sb = res.tile([128, KC, H * 65], BF16, tag="v")   # V+ones [k, h*65+d]
        ctxn_sb = res.tile([128, IC, SQ], BF16, tag="ctxn")  # ctx^T [c, q]
        negid_sb = res.tile([128, 128], FP8, tag="negid")   # MASK_C * I
        wo_sb = res.tile([128, IC, D], BF16, tag="wo")
        xr_sb = res.tile([128, SQ // 128, D], F32, tag="xr")
        eps_sb = res.tile([128, 1], F32, tag="eps")

        # ---- phase 1: projections ----
        with tc.tile_pool(name="p1", bufs=1) as p1:
            xq_sb = p1.tile([128, IC, SQ], BF16, tag="xq")
            xk_sb = p1.tile([128, IC, S], BF16, tag="xk")
            wk_sb = p1.tile([128, IC, D], BF16, tag="wk")
            wv_sb = p1.tile([128, IC, D], BF16, tag="wv")
            # DMA order: the Q/K inputs the first matmuls need come first.
            nc.sync.dma_start(xq_sb[:], xqT.rearrange("(ic p) q -> p ic q", p=128))
            for nt in range(4):
                nc.sync.dma_start(
                    xk_sb[:, :, nt * 512:(nt + 1) * 512],
                    xkT[:, nt * 512:(nt + 1) * 512]
                    .rearrange("(ic p) k -> p ic k", p=128))
            for oc in range(4):
                nc.sync.dma_start(
                    wk_sb[:, :, oc * 256:(oc + 1) * 256],
                    w_k[:, oc * 256:(oc + 1) * 256]
                    .rearrange("(ic p) o -> p ic o", p=128))
            # ones for the V ones-column (V copies overwrite the rest)
            nc.vector.memset(v_sb[:], 1.0)
            nc.vector.memset(eps_sb[:], LN_EPS)

            # Q^T and K^T per head-pair (pair 0 finishes first so attention
            # can start while later pairs still project).
            for hpp in range(IC // 2):
                wq_t = p1.tile([128, IC, 256], BF16, tag="wq", bufs=2)
                nc.sync.dma_start(
                    wq_t[:],
                    w_q[:, hpp * 256:(hpp + 1) * 256]
                    .rearrange("(ic p) o -> p ic o", p=128))
                ps_q = psum.tile([128, 1024], F32, tag="mm2", bufs=3)
                for j2 in range(2):
                    for ic in range(IC):
                        nc.tensor.matmul(ps_q[:, j2 * 512:(j2 + 1) * 512],
                                         lhsT=wq_t[:, ic, j2 * 128:(j2 + 1) * 128],
                                         rhs=xq_sb[:, ic, :],
                                         start=(ic == 0), stop=(ic == IC - 1))
                nc.scalar.copy(qt_sb[:, 2 * hpp:2 * hpp + 2, :],
                               ps_q[:].rearrange("p (x q) -> p x q", x=2))
                for nt in range(S // 512):
                    ps_k = psum.tile([128, 1024], F32, tag="mm2", bufs=3)
                    for j2 in range(2):
                        hp = 2 * hpp + j2
                        for ic in range(IC):
                            nc.tensor.matmul(
                                ps_k[:, j2 * 512:(j2 + 1) * 512],
                                lhsT=wk_sb[:, ic, hp * 128:(hp + 1) * 128],
                                rhs=xk_sb[:, ic, nt * 512:(nt + 1) * 512],
                                start=(ic == 0), stop=(ic == IC - 1))
                    nc.scalar.copy(
                        kt_sb[:, 2 * hpp:2 * hpp + 2, nt * 512:(nt + 1) * 512],
                        ps_k[:].rearrange("p (x q) -> p x q", x=2))

            nc.sync.dma_start(negid_sb[:], negid)
            nc.sync.dma_start(wv_sb[:], w_v.rearrange("(ic p) o -> p ic o", p=128))
            nc.sync.dma_start(wo_sb[:], w_o.rearrange("(ic p) o -> p ic o", p=128))
            nc.sync.dma_start(xr_sb[:], xq_res.rearrange("(qc p) o -> p qc o", p=128))

            # V[k, o]: lhsT = x_v^T k-block (streamed), rhs = W_V. Strided copy
            # into v_sb leaves the per-head ones-columns (h*65+64) intact.
            for kc in range(KC):
                xv_t = p1.tile([128, IC, 128], BF16, tag="xv", bufs=2)
                nc.sync.dma_start(
                    xv_t[:],
                    xvT[:, kc * 128:(kc + 1) * 128]
                    .rearrange("(ic p) k -> p ic k", p=128))
                ps_v = psum.tile([128, 1024], F32, tag="mm2", bufs=3)
                for half in range(2):
                    for ic in range(IC):
                        nc.tensor.matmul(ps_v[:, half * 512:(half + 1) * 512],
                                         lhsT=xv_t[:, ic, :],
                                         rhs=wv_sb[:, ic, half * 512:(half + 1) * 512],
                                         start=(ic == 0), stop=(ic == IC - 1))
                dst = v_sb[:, kc, :].rearrange("p (h x) -> p h x", x=65)
                nc.scalar.copy(dst[:, :, 0:64],
                               ps_v[:].rearrange("p (h d) -> p h d", d=64))

        # ---- phase 2: attention, head pairs (row-packed K=64 matmuls) ----
        # Scores go into 2-bank PSUM tiles; the mask is applied by a second
        # accumulating matmul with lhsT = -1e9*I and rhs = maskT, so exp can
        # read fully-masked scores and no elementwise mask op is needed.
        with tc.tile_pool(name="p2", bufs=1) as p2:
            mask_sb = p2.tile([128, KC, SQ], FP8, tag="mask")  # 1.0 = blocked
            nc.sync.dma_start(mask_sb[:],
                              maskT.rearrange("(kc p) q -> p kc q", p=128))
            for hp in range(IC):
                heads = (2 * hp, 2 * hp + 1)
                masked = [p2.tile([128, KC, SQ], BF16, tag="masked", bufs=3,
                                  name=f"masked_{h}") for h in heads]
                for kcg in range(KC // 2):
                    ps_s = [psum.tile([128, 1024], F32, tag="mm2", bufs=3,
                                      name=f"ps_s_{j}") for j in range(2)]
                    # the two K=64 QK matmuls sit in different PE row groups
                    # and run concurrently when issued back-to-back
                    for t in range(2):
                        kc = 2 * kcg + t
                        for j in range(2):
                            p0 = j * 64
                            nc.tensor.matmul(
                                ps_s[j][:, t * 512:(t + 1) * 512],
                                lhsT=kt_sb[p0:p0 + 64, hp,
                                           kc * 128:(kc + 1) * 128],
                                rhs=qt_sb[p0:p0 + 64, hp, :],
                                start=True, stop=False)
                    for t in range(2):
                        kc = 2 * kcg + t
                        for j in range(2):
                            nc.tensor.matmul(
                                ps_s[j][:, t * 512:(t + 1) * 512],
                                lhsT=negid_sb[:],
                                rhs=mask_sb[:, kc, :],
                                start=False, stop=True)
                    for j in range(2):
                        nc.scalar.activation(
                            masked[j][:, 2 * kcg:2 * kcg + 2, :], ps_s[j][:],
                            mybir.ActivationFunctionType.Exp, scale=0.125)
                # attn @ [V | 1]: ctx^T rows 0..63, denominators in row 64.
                # Both heads share one 2-bank psum tile so recip + broadcast
                # run once per pair.
                pc = psum.tile([65, 1024], F32, tag="ctx", bufs=1)
                for j, h in enumerate(heads):
                    for kc in range(KC):
                        nc.tensor.matmul(pc[:, j * 512:(j + 1) * 512],
                                         lhsT=v_sb[:, kc, h * 65:(h + 1) * 65],
                                         rhs=masked[j][:, kc, :],
                                         start=(kc == 0), stop=(kc == KC - 1))
                recip = p2.tile([1, 1024], BF16, tag="recip", bufs=1)
                with nc.allow_low_precision(reason="softmax denom recip to bf16"):
                    nc.vector.reciprocal(recip[:], pc[64:65, :])
                bcast = p2.tile([128, 1024], BF16, tag="bcast", bufs=2)
                nc.gpsimd.partition_broadcast(bcast[:], recip[:])
                for j, h in enumerate(heads):
                    p0 = j * 64
                    sl = slice(j * 512, (j + 1) * 512)
                    bc8 = bcast[:, sl].rearrange("p (x q) -> p x q", x=1) \
                        .to_broadcast((128, 8, 512))
                    for g in range(2):
                        at = p2.tile([128, 8, SQ], F32, tag="attn_out", bufs=2)
                        nc.vector.tensor_tensor(
                            at[:], masked[j][:, g * 8:(g + 1) * 8, :], bc8,
                            op=mybir.AluOpType.mult)
                        nc.sync.dma_start(
                            attn_t[h, g * 1024:(g + 1) * 1024, :]
                            .rearrange("(x p) q -> p x q", p=128),
                            at[:])
                    nc.vector.tensor_tensor(ctxn_sb[p0:p0 + 64, hp, :],
                                            pc[0:64, sl], bcast[0:64, sl],
                                            op=mybir.AluOpType.mult)

        # ---- phase 3: output projection + residual + layernorm ----
        with tc.tile_pool(name="p3", bufs=1) as p3:
            for qc in range(SQ // 128):
                ps_y = psum.tile([128, 1024], F32, tag="mm2", bufs=3)
                for half in range(2):
                    for cc in range(IC):
                        nc.tensor.matmul(
                            ps_y[:, half * 512:(half + 1) * 512],
                            lhsT=ctxn_sb[:, cc, qc * 128:(qc + 1) * 128],
                            rhs=wo_sb[:, cc, half * 512:(half + 1) * 512],
                            start=(cc == 0), stop=(cc == IC - 1))
                y_sb = p3.tile([128, D], F32, tag="y", bufs=2)
                nc.vector.tensor_add(y_sb[:], ps_y[:], xr_sb[:, qc, :])
                nmean = p3.tile([128, 1], F32, tag="nmean", bufs=2)
                nc.vector.reduce_sum(nmean[:], y_sb[:], axis=mybir.AxisListType.X)
                nc.vector.tensor_scalar_mul(nmean[:], nmean[:], -1.0 / D)
                nc.vector.tensor_scalar_add(y_sb[:], y_sb[:], nmean[:])
                sq_ps = psum.tile([128, 1024], F32, tag="mm2", bufs=3)
                var = p3.tile([128, 1], F32, tag="var", bufs=2)
                nc.scalar.activation(sq_ps[:], y_sb[:],
                                     mybir.ActivationFunctionType.Square,
                                     accum_out=var[:])
                std = p3.tile([128, 1], F32, tag="std", bufs=2)
                nc.scalar.activation(std[:], var[:],
                                     mybir.ActivationFunctionType.Sqrt,
                                     bias=eps_sb[:], scale=1.0 / D)
                rstd = p3.tile([128, 1], F32, tag="rstd", bufs=2)
                nc.vector.reciprocal(rstd[:], std[:])
                o_sb = p3.tile([128, D], F32, tag="o", bufs=2)
                nc.vector.tensor_scalar_mul(o_sb[:], y_sb[:], rstd[:])
                nc.sync.dma_start(y_out[qc * 128:(qc + 1) * 128, :], o_sb[:])

    nc.compile()
    return nc


_NC_CACHE = None


def get_nc():
    global _NC_CACHE
    if _NC_CACHE is None:
        _NC_CACHE = build()
    return _NC_CACHE


def make_in_maps(input_Q, input_K, input_V, attn_mask, W_Q, W_K, W_V, W_O):
    bf = ml_dtypes.bfloat16
    input_Q = np.asarray(input_Q, np.float32)
    input_K = np.asarray(input_K, np.float32)
    input_V = np.asarray(input_V, np.float32)
    attn_mask = np.asarray(attn_mask, bool)
    shared = {
        "w_q": np.asarray(W_Q, np.float32).astype(bf),
        "w_k": np.asarray(W_K, np.float32).astype(bf),
        "w_v": np.asarray(W_V, np.float32).astype(bf),
        "w_o": np.asarray(W_O, np.float32).astype(bf),
        "negid": (np.eye(128, dtype=np.float32) * MASK_C).astype(
            ml_dtypes.float8_e4m3fn),
    }
    per_batch = {}
    for b in range(B):
        per_batch[b] = {
            "xkT": np.ascontiguousarray(input_K[b].T).astype(bf),
            "xvT": np.ascontiguousarray(input_V[b].T).astype(bf),
        }
    in_maps = []
    for c in range(N_CORES):
        b, qi = divmod(c, 4)
        q0 = qi * SQ
        xq = input_Q[b, q0:q0 + SQ, :]
        in_maps.append({
            **shared,
            **per_batch[b],
            "xqT": np.ascontiguousarray(xq.T).astype(bf),
            "xq_res": np.ascontiguousarray(xq),
            "maskT": np.ascontiguousarray(
                attn_mask[b, q0:q0 + SQ, :].T.astype(np.float32)).astype(
                    ml_dtypes.float8_e4m3fn),
        })
    return in_maps


def assemble(results):
    out = np.empty((B, S, D), np.float32)
    attn = np.empty((B, H, S, S), np.float32)
    for c in range(N_CORES):
        b, qi = divmod(c, 4)
        q0 = qi * SQ
        out[b, q0:q0 + SQ] = results[c]["y_out"]
        attn[b, :, q0:q0 + SQ, :] = results[c]["attn_t"].transpose(0, 2, 1)
    return out, attn


def run(inputs, trace=False, trace_kwargs=None):
    from concourse.bass_utils import run_bass_kernel_spmd
    nc = get_nc()
    in_maps = make_in_maps(**inputs)
    res = run_bass_kernel_spmd(nc, in_maps, core_ids=list(range(N_CORES)),
                               trace=trace, **(trace_kwargs or {}))
    return res


def kernel(**inputs):
    res = run(inputs, trace=False)
    return assemble(res.results)
